# revision 1
# baseline (speedup 1.0000x reference)
"""AttentionAggregator kernel for 8 Trainium2 NeuronCores.

Math reformulation (exact):
  score[b,k] = leakyrelu(feat[nb[b,k]]@w1 + feat[node[b]]@w2),  w1 = kernel1[0]@aw[:D], w2 = kernel[0]@aw[D:]
  p = softmax_k(score);  out = (sum_k p[b,k]*feat[nb[b,k]]) @ (kernel1[0] @ neigh_weights)

Device work per core (B/8 = 2500 targets, 80000 neighbour-row gathers):
  - 16 dma_gather calls (int16 indices over 31250-row windows of the feature table)
  - per-slot dot products: fused multiply+running-sum scan on DVE, boundary diffs
  - e = exp(leakyrelu(sigma + node_term)) on ACT
  - per-target aggregation: PE matmuls with host-built one-hot (target-selection)
    weight matrices, accumulated in PSUM across all 16 segment sweeps
  - normalize by softmax denominator, project by Wout on PE, write out
Host only prepares index/selection/metadata tensors and the (cheap) node terms.
"""

import sys

sys.path.insert(0, "/opt/trn_rl_repo")

import numpy as np

N_NODES = 500000
D = 128
K = 32
B = 20000
NCORES = 8
BPC = B // NCORES          # 2500 targets per core
NSEG = 16
SEGROWS = N_NODES // NSEG  # 31250 rows per index window (< 32768 int16 limit)
NGRP = 46                  # target groups per core
GCAP = 64                  # max targets per group (lhsT width)
SCAP = 128                 # max slots per (group, segment) -> one 128-slot column
COLS_PER_SEG = NGRP        # one column per group per segment
NCOLS = NSEG * COLS_PER_SEG            # 736 columns of 128 slots per core
NSLOT = NCOLS * 128                    # 94208 slots per core
OUTROWS = NGRP * GCAP                  # 2944 padded output rows per core
NT_PAD = -1.0e4


def _pack_groups(cnt):
    """Pack BPC targets into NGRP groups (<=GCAP targets, per-seg slots <=SCAP).

    cnt: [BPC, NSEG] int edge counts per target per segment.
    Returns list of lists of target ids.
    """
    order = np.argsort(-cnt.max(axis=1), kind="stable")
    sums = np.zeros((NGRP, NSEG), np.int64)
    sizes = np.zeros(NGRP, np.int64)
    assign = np.full(BPC, -1, np.int64)
    for t in order:
        c = cnt[t]
        ok = (sizes < GCAP) & np.all(sums + c <= SCAP, axis=1)
        cand = np.nonzero(ok)[0]
        if len(cand) == 0:
            raise RuntimeError("group packing failed; raise NGRP")
        # best-fit: fullest feasible group first
        g = cand[np.argmax(sizes[cand])]
        assign[t] = g
        sums[g] += c
        sizes[g] += 1
    groups = [list(np.nonzero(assign == g)[0]) for g in range(NGRP)]
    return groups


def _prepare_core(nb_local, nt_local):
    """Build per-core device tensors.

    nb_local: [BPC, K] int32 global neighbour ids
    nt_local: [BPC] float32 node terms
    Returns dict of arrays + row_map (packed out row -> local target or -1).
    """
    seg = nb_local // SEGROWS           # [BPC, K]
    loc = nb_local % SEGROWS
    cnt = np.zeros((BPC, NSEG), np.int64)
    for s in range(NSEG):
        cnt[:, s] = (seg == s).sum(axis=1)
    groups = _pack_groups(cnt)

    idx16 = np.zeros((128, NCOLS * 8), np.int16)   # wrapped: [128, nslots/16]
    ntt = np.full((128, NCOLS), NT_PAD, np.float32)
    S = np.zeros((128, NCOLS * GCAP), np.uint8)
    row_map = np.full(OUTROWS, -1, np.int64)

    idx_flat = np.zeros(NSLOT, np.int16)
    for g, tlist in enumerate(groups):
        for r, t in enumerate(tlist):
            row_map[g * GCAP + r] = t
        tl = np.array(tlist, np.int64)
        segs_g = seg[tl]                # [len, K]
        locs_g = loc[tl]
        for s in range(NSEG):
            col = s * COLS_PER_SEG + g
            base = col * 128
            pos = 0
            for r, t in enumerate(tl):
                ks = np.nonzero(segs_g[r] == s)[0]
                n = len(ks)
                if n == 0:
                    continue
                sl = slice(base + pos, base + pos + n)
                idx_flat[sl] = locs_g[r, ks].astype(np.int16)
                p0 = np.arange(pos, pos + n)
                ntt[p0, col] = nt_local[t]
                S[p0, col * GCAP + r] = 1
                pos += n
            assert pos <= 128
    # wrap indices: idx16[p, w] = idx_flat[w*16 + p%16], replicated across 8 groups
    w = NSLOT // 16
    wrapped = idx_flat.reshape(w, 16).T          # [16, w]
    idx16[:, :] = np.tile(wrapped, (8, 1))
    return dict(idx16=idx16, ntt=ntt, S=S), row_map


_CACHE = {}


def _build_program():
    import concourse.bacc as bacc
    import concourse.bass as bass
    import concourse.mybir as mybir
    import concourse.tile as tile
    from concourse.masks import make_identity

    nc = bacc.Bacc("TRN2", target_bir_lowering=False, debug=False,
                   num_devices=NCORES)
    dt = mybir.dt
    feat_d = nc.dram_tensor("feat", [N_NODES, D], dt.float32, kind="ExternalInput")
    idx_d = nc.dram_tensor("idx16", [128, NCOLS * 8], dt.int16, kind="ExternalInput")
    nt_d = nc.dram_tensor("ntt", [128, NCOLS], dt.float32, kind="ExternalInput")
    s_d = nc.dram_tensor("S", [128, NCOLS * GCAP], dt.uint8, kind="ExternalInput")
    w1_d = nc.dram_tensor("w1rep", [128, D], dt.float32, kind="ExternalInput")
    wo_d = nc.dram_tensor("Wout", [128, D], dt.float32, kind="ExternalInput")
    o_d = nc.dram_tensor("o", [OUTROWS, D], dt.float32, kind="ExternalOutput")

    CS = COLS_PER_SEG                  # 46 cols per segment call
    SLOTS_S = CS * 128                 # 5888 slots per call

    import os
    DEBUG = bool(int(os.environ.get("KDBG", "0")))
    if DEBUG:
        dbg_g = nc.dram_tensor("dbg_g", [128, SLOTS_S], dt.float32, kind="ExternalOutput")
        dbg_sig = nc.dram_tensor("dbg_sig", [128, CS], dt.float32, kind="ExternalOutput")
        dbg_et = nc.dram_tensor("dbg_et", [128, CS], dt.float32, kind="ExternalOutput")
        dbg_wt = nc.dram_tensor("dbg_wt", [128, CS * GCAP], dt.float32, kind="ExternalOutput")
        dbg_w1b = nc.dram_tensor("dbg_w1b", [128, CS * D], dt.float32, kind="ExternalOutput")
        dbg_acc = nc.dram_tensor("dbg_acc", [128, D], dt.float32, kind="ExternalOutput")
        dbg_z = nc.dram_tensor("dbg_z", [128, 1], dt.float32, kind="ExternalOutput")
        dbg_an = nc.dram_tensor("dbg_an", [128, D], dt.float32, kind="ExternalOutput")
        dbg_ant = nc.dram_tensor("dbg_ant", [128, D], dt.float32, kind="ExternalOutput")

    with tile.TileContext(nc) as tc:
        with (
            tc.tile_pool(name="big", bufs=2) as big,
            tc.tile_pool(name="small", bufs=2) as small,
            tc.tile_pool(name="persist", bufs=1) as persist,
            tc.tile_pool(name="psum", bufs=1, space="PSUM") as psump,
            tc.tile_pool(name="epi", bufs=2) as epi,
            tc.tile_pool(name="episum", bufs=1, space="PSUM") as episum,
        ):
            w1t = persist.tile([128, D], dt.float32)
            wot = persist.tile([128, D], dt.float32)
            onest = persist.tile([128, 1], dt.float32)
            ident = persist.tile([128, 128], dt.float32)
            nc.sync.dma_start(out=w1t[:], in_=w1_d[:, :])
            nc.sync.dma_start(out=wot[:], in_=wo_d[:, :])
            nc.vector.memset(onest[:], 1.0)
            make_identity(nc, ident[:])
            # w1 tiled CS times along free dim (scan operands must be 2D)
            w1big = persist.tile([128, CS * D], dt.float32)
            _w1v = w1t[:]
            nc.vector.tensor_copy(
                out=w1big[:].rearrange("p (c d) -> p c d", d=D),
                in_=bass.AP(_w1v.tensor, _w1v.offset,
                            [_w1v.ap[0], [0, CS], [1, D]]),
            )

            # persistent PSUM accumulators: 23 pair-accs of [128, 128] packed
            # 4-per-bank into 6 banks, plus one bank of Z columns.
            accbanks = [psump.tile([128, 512], dt.float32, tag=f"accb{i}", name=f"accb{i}")
                        for i in range(6)]
            zbank = psump.tile([128, 512], dt.float32, tag="zbank", name="zbank")

            def acc_n(pair):   # [128, 128] slice for pair's N accumulator
                return accbanks[pair // 4][:, (pair % 4) * 128:(pair % 4 + 1) * 128]

            def acc_z(pair):   # [128, 1] slice for pair's Z accumulator
                return zbank[:, pair:pair + 1]

            # start=True clears the WHOLE psum bank, so banks shared by
            # several accumulators are zeroed once up front and every real
            # matmul accumulates (start=False writes where has_written=0).
            zerot = persist.tile([128, 128], dt.float32)
            nc.vector.memset(zerot[:], 0.0)
            for bank in accbanks + [zbank]:
                nc.tensor.matmul(out=bank[:, :], lhsT=zerot[:],
                                 rhs=w1big[:, :512], start=True, stop=False,
                                 skip_group_check=True)

            for s in range(NSEG):
                idx_t = small.tile([128, CS * 8], dt.int16, tag="idx")
                ntc = small.tile([128, CS], dt.float32, tag="nt")
                sc = big.tile([128, CS * GCAP], dt.uint8, tag="S")
                nc.sync.dma_start(out=idx_t[:], in_=idx_d[:, s * CS * 8:(s + 1) * CS * 8])
                nc.sync.dma_start(out=ntc[:], in_=nt_d[:, s * CS:(s + 1) * CS])
                nc.sync.dma_start(out=sc[:], in_=s_d[:, s * CS * GCAP:(s + 1) * CS * GCAP])

                g = big.tile([128, SLOTS_S], dt.float32, tag="g")
                nc.gpsimd.dma_gather(
                    out_ap=g[:].rearrange("p (c d) -> p c d", d=D),
                    in_ap=feat_d[s * SEGROWS:N_NODES, :],
                    idxs_ap=idx_t[:],
                    num_idxs=SLOTS_S,
                    num_idxs_reg=SLOTS_S,
                    elem_size=D,
                    single_packet=False,
                )

                # per-slot dot with w1: elementwise mul + segmented reduce
                prod = big.tile([128, SLOTS_S], dt.float32, tag="prod")
                nc.vector.tensor_tensor(out=prod[:], in0=g[:], in1=w1big[:],
                                        op=mybir.AluOpType.mult)
                sig = small.tile([128, CS], dt.float32, tag="sig")
                nc.vector.tensor_reduce(
                    out=sig[:],
                    in_=prod[:].rearrange("p (c d) -> p c d", d=D),
                    axis=mybir.AxisListType.X,
                    op=mybir.AluOpType.add,
                )
                nc.vector.tensor_tensor(out=sig[:], in0=sig[:], in1=ntc[:],
                                        op=mybir.AluOpType.add)
                lr = small.tile([128, CS], dt.float32, tag="lr")
                nc.vector.tensor_scalar_mul(lr[:], sig[:], 0.2)
                nc.vector.tensor_tensor(out=lr[:], in0=lr[:], in1=sig[:],
                                        op=mybir.AluOpType.max)
                et = small.tile([128, CS], dt.float32, tag="et")
                nc.scalar.activation(et[:], lr[:], mybir.ActivationFunctionType.Exp)

                wt = big.tile([128, CS * GCAP], dt.float32, tag="W")
                ev = et[:]
                ebc = bass.AP(ev.tensor, ev.offset,
                              [ev.ap[0], [1, CS], [0, GCAP]])
                nc.vector.tensor_tensor(
                    out=wt[:].rearrange("p (c w) -> p c w", w=GCAP),
                    in0=sc[:].rearrange("p (c w) -> p c w", w=GCAP),
                    in1=ebc,
                    op=mybir.AluOpType.mult,
                )

                if DEBUG and s == 0:
                    nc.sync.dma_start(out=dbg_g[:, :], in_=g[:])
                    nc.sync.dma_start(out=dbg_sig[:, :], in_=sig[:])
                    nc.sync.dma_start(out=dbg_et[:, :], in_=et[:])
                    nc.sync.dma_start(out=dbg_wt[:, :], in_=wt[:])
                    nc.sync.dma_start(out=dbg_w1b[:, :], in_=w1big[:])

                for gi in range(NGRP):
                    pair, off = gi // 2, (gi % 2) * GCAP
                    last = (s == NSEG - 1)
                    nc.tensor.matmul(
                        out=acc_n(pair)[off:off + GCAP, :],
                        lhsT=wt[:, gi * GCAP:(gi + 1) * GCAP],
                        rhs=g[:, gi * D:(gi + 1) * D],
                        start=False, stop=last, skip_group_check=True,
                    )
                    nc.tensor.matmul(
                        out=acc_z(pair)[off:off + GCAP, :],
                        lhsT=wt[:, gi * GCAP:(gi + 1) * GCAP],
                        rhs=onest[:],
                        start=False, stop=last, skip_group_check=True,
                    )

            for pair in range(NGRP // 2):
                acc = epi.tile([128, D], dt.float32, tag="acc_sb")
                nc.vector.tensor_copy(out=acc[:], in_=acc_n(pair))
                zsb = epi.tile([128, 1], dt.float32, tag="zsb")
                nc.vector.tensor_copy(out=zsb[:], in_=acc_z(pair))
                rcp = epi.tile([128, 1], dt.float32, tag="rcp")
                nc.vector.reciprocal(rcp[:], zsb[:])
                an = epi.tile([128, D], dt.float32, tag="an")
                nc.vector.tensor_scalar_mul(an[:], acc[:], rcp[:])
                pst = episum.tile([128, D], dt.float32, tag="eps")
                nc.tensor.transpose(out=pst[:], in_=an[:], identity=ident[:])
                ant = epi.tile([128, D], dt.float32, tag="ant")
                nc.vector.tensor_copy(out=ant[:], in_=pst[:])
                pso = episum.tile([128, D], dt.float32, tag="eps")
                nc.tensor.matmul(out=pso[:], lhsT=ant[:], rhs=wot[:],
                                 start=True, stop=True)
                osb = epi.tile([128, D], dt.float32, tag="osb")
                nc.vector.tensor_copy(out=osb[:], in_=pso[:])
                nc.sync.dma_start(out=o_d[pair * 128:(pair + 1) * 128, :], in_=osb[:])
                if DEBUG and pair == 0:
                    nc.sync.dma_start(out=dbg_acc[:, :], in_=acc[:])
                    nc.sync.dma_start(out=dbg_z[:, :], in_=zsb[:])
                    nc.sync.dma_start(out=dbg_an[:, :], in_=an[:])
                    nc.sync.dma_start(out=dbg_ant[:, :], in_=ant[:])

    nc.compile()
    return nc


def kernel(features, node, neighbours, kernel, kernel1, attention_weights,
           neigh_weights):
    from concourse.bass_utils import run_bass_kernel_spmd

    features = np.asarray(features, np.float32)
    node = np.asarray(node, np.int32)
    neighbours = np.asarray(neighbours, np.int32)
    kernel = np.asarray(kernel, np.float32)
    kernel1 = np.asarray(kernel1, np.float32)
    attention_weights = np.asarray(attention_weights, np.float32)
    neigh_weights = np.asarray(neigh_weights, np.float32)

    a1 = attention_weights[0, :D]
    a2 = attention_weights[0, D:]
    w1 = kernel1[0] @ a1                       # [D]
    w2 = kernel[0] @ a2                        # [D]
    wout = kernel1[0] @ neigh_weights          # [D, UNITS]
    nt_all = features[node[:, 0]] @ w2         # [B] node terms (host)

    w1rep = np.tile(w1[None, :], (128, 1)).astype(np.float32)
    wout = np.ascontiguousarray(wout, np.float32)

    in_maps = []
    row_maps = []
    for c in range(NCORES):
        nb = neighbours[c * BPC:(c + 1) * BPC]
        nt = nt_all[c * BPC:(c + 1) * BPC].astype(np.float32)
        t, rmap = _prepare_core(nb, nt)
        row_maps.append(rmap)
        in_maps.append({
            "feat": features,
            "idx16": t["idx16"],
            "ntt": t["ntt"],
            "S": t["S"],
            "w1rep": w1rep,
            "Wout": wout,
        })

    if "nc" not in _CACHE:
        _CACHE["nc"] = _build_program()
    nc = _CACHE["nc"]

    res = run_bass_kernel_spmd(nc, in_maps, core_ids=list(range(NCORES)))
    out = np.zeros((B, D), np.float32)
    for c in range(NCORES):
        oc = res.results[c]["o"]               # [OUTROWS, D]
        rmap = row_maps[c]
        valid = rmap >= 0
        out[c * BPC + rmap[valid]] = oc[valid]
    return out

